# revision 29
# baseline (speedup 1.0000x reference)
"""Per-pixel depthwise 3x3 conv (Conv2dLocal) on 8 Trainium2 NeuronCores.

out[b,c,h,w] = sum_{i,j in 3x3} x[b,c,h+i-1,w+j-1] * weight[b, c*9+3i+j, h, w]

Sharding: 8 cores = 2 batches x 4 H-slabs of 64 rows (data/spatial parallel).
The host pads the input spatially (1-px halo on H and W) and hands every core
an overlapping x slab, so the device program is identical and branch-free on
all cores (pure SPMD, no collectives).

The kernel is HBM-bandwidth-bound on the weight stream (per-pixel 3x3 taps =
9 weight elements per output element). The correctness budget (rel err 2e-2)
is far looser than fp32, so all device I/O and the multiplies run in fp16:
half the bytes of the fp32 version, and DVE's 2x mode (2-byte dtypes, packed
innermost dim) doubles multiply throughput. Accuracy stays ~1e-3 because the
9-tap accumulation happens in fp32 PSUM via identity-matmul on the PE (fp16
matmul streams at full rate), and only the final result rounds back to fp16.

Per-core layout: partition p = hb*32 + c (hb: 16-row block 0..3, c: channel);
free dim = (row, w), so all nine 3x3 tap shifts are free-dim views of a
single resident x slab [128, 18, 514].

Engines: DVE does the 9 per-tap multiplies (fp16 2x mode, reading shifted
x views straight from SBUF); PE accumulates all 9 products per 512-col row
chunk into an fp32 PSUM bank (interleaved across 4 banks per group, so PE
consumes each product right after DVE emits it); ScalarE drains PSUM to SBUF
with the fp32->fp16 downcast and issues x/output DMAs; the sync ring streams
the 36 weight tiles, each a contiguous 512 KiB DRAM block (contiguous-source
DMAs stripe over all 16 SDMA engines).
"""

import sys

if "/opt/trn_rl_repo" not in sys.path:
    sys.path.insert(0, "/opt/trn_rl_repo")

from contextlib import ExitStack

import numpy as np

import concourse.mybir as mybir
import concourse.tile as tile
from concourse import bacc
from concourse.bass_utils import run_bass_kernel_spmd
from concourse.masks import make_identity

# Problem shape (hardcoded per harness contract)
B, C, H, W = 2, 32, 256, 512
K = 3
KK = K * K
N_CORES = 8

# Per-core decomposition
HL = H // 4          # 64 local rows per core
HB = 4               # row-blocks per core (partition groups)
RB = HL // HB        # 16 rows per partition
G = 4                # rows processed per group
NGRP = RB // G       # 4 groups
WP = W + 2           # width incl. halo
NP = 128             # partitions

FP16 = mybir.dt.float16
FP32 = mybir.dt.float32
MULT = mybir.AluOpType.mult

_PROGRAM = None


def _build_program() -> bacc.Bacc:
    nc = bacc.Bacc(
        "TRN2", target_bir_lowering=False, debug=False, num_devices=N_CORES
    )
    x_d = nc.declare_dram_parameter("x", [HB, C, RB + 2, WP], FP16, isOutput=False)
    w_d = nc.declare_dram_parameter(
        "w", [KK, NGRP, HB, C, G, W], FP16, isOutput=False
    )
    o_d = nc.declare_dram_parameter("o", [NGRP, HB, C, G, W], FP16, isOutput=True)

    with tile.TileContext(nc) as tc, ExitStack() as ctx:
        x_pool = ctx.enter_context(tc.tile_pool(name="x", bufs=1))
        w_pool = ctx.enter_context(tc.tile_pool(name="wt", bufs=16))
        prod_pool = ctx.enter_context(tc.tile_pool(name="prod", bufs=6))
        out_pool = ctx.enter_context(tc.tile_pool(name="outsb", bufs=3))
        const_pool = ctx.enter_context(tc.tile_pool(name="const", bufs=1))
        pe_pool = ctx.enter_context(tc.tile_pool(name="pe", bufs=8, space="PSUM"))

        # Prefetch the first weight tile on the gpsimd (SWDGE) ring: GpSimd's
        # preamble finishes ~1.5us before the sync ring issues its first
        # config, so this recovers otherwise-idle DMA-pool time at the ramp
        # and shortens the sync weight stream by one tile.
        w00 = w_pool.tile([NP, G, W], FP16, tag="wt", name="w00")
        nc.gpsimd.dma_start(out=w00, in_=w_d[0, 0])

        ident = const_pool.tile([NP, NP], FP16)
        make_identity(nc, ident)

        # resident x slab: per partition 18 rows (16 + 2 halo) x 514 cols.
        # Split the load so the first group's rows land early (faster ramp).
        x_sb = x_pool.tile([NP, RB + 2, WP], FP16)
        nc.scalar.dma_start(out=x_sb[:, 0 : G + 2, :], in_=x_d[:, :, 0 : G + 2, :])
        nc.scalar.dma_start(
            out=x_sb[:, G + 2 : RB + 2, :], in_=x_d[:, :, G + 2 : RB + 2, :]
        )

        for grp in range(NGRP):
            R = grp * G
            last_grp = grp == NGRP - 1
            pe_banks = [
                pe_pool.tile([NP, W], FP32, tag="peps", name=f"pe_{grp}_{c}")
                for c in range(G)
            ]
            for t in range(KK):
                i, j = t // K, t % K
                if grp == 0 and t == 0:
                    wt = w00
                else:
                    wt = w_pool.tile([NP, G, W], FP16, tag="wt")
                    nc.sync.dma_start(out=wt, in_=w_d[t, grp])
                prod = prod_pool.tile([NP, G, W], FP16, tag="prod")
                nc.vector.tensor_tensor(
                    prod[:], wt[:], x_sb[:, R + i : R + i + G, j : j + W], MULT
                )
                for c in range(G):
                    nc.tensor.matmul(
                        pe_banks[c][:],
                        ident[:],
                        prod[:, c, :],
                        start=(t == 0),
                        stop=(t == KK - 1),
                    )
            out_sb = out_pool.tile([NP, G, W], FP16, tag="outsb")
            if not last_grp:
                for c in range(G):
                    nc.scalar.copy(out=out_sb[:, c, :], in_=pe_banks[c][:])
            else:
                # Tail: DVE is idle after the final multiply, so it drains
                # half the PSUM banks in parallel with ScalarE.
                nc.scalar.copy(out=out_sb[:, 0, :], in_=pe_banks[0][:])
                nc.vector.tensor_copy(out=out_sb[:, 1, :], in_=pe_banks[1][:])
                nc.scalar.copy(out=out_sb[:, 2, :], in_=pe_banks[2][:])
                nc.vector.tensor_copy(out=out_sb[:, 3, :], in_=pe_banks[3][:])
            nc.scalar.dma_start(out=o_d[grp], in_=out_sb[:])

    nc.compile()
    return nc


def _get_program() -> bacc.Bacc:
    global _PROGRAM
    if _PROGRAM is None:
        _PROGRAM = _build_program()
    return _PROGRAM


def _shard_inputs(input: np.ndarray, weight: np.ndarray) -> list[dict]:
    xp = np.pad(input, ((0, 0), (0, 0), (1, 1), (1, 1)))
    in_maps = []
    for k in range(N_CORES):
        b, hb = k // 4, k % 4
        h0 = hb * HL
        xs = xp[b, :, h0 : h0 + HL + 2, :]  # [C, 66, WP]
        # x: the HB overlapping 18-row windows -> [HB, C, 18, WP]
        x4 = np.stack(
            [xs[:, r0 : r0 + RB + 2, :] for r0 in range(0, HL, RB)]
        ).astype(np.float16)
        # weights: [C*KK, HL, W] -> [tap, grp, hb, c, r, w], contiguous per
        # (tap, grp) so each device DMA reads one linear 512 KiB block
        ws = (
            weight[b]
            .reshape(C, KK, H, W)[:, :, h0 : h0 + HL, :]
            .reshape(C, KK, HB, NGRP, G, W)
            .transpose(1, 3, 2, 0, 4, 5)
        )
        ws = np.ascontiguousarray(ws, dtype=np.float16)
        in_maps.append({"x": x4, "w": ws})
    return in_maps


def kernel(input: np.ndarray, weight: np.ndarray, _trace: bool = False):
    nc = _get_program()
    in_maps = _shard_inputs(np.asarray(input), np.asarray(weight))
    res = run_bass_kernel_spmd(
        nc, in_maps, core_ids=list(range(N_CORES)), trace=_trace
    )
    out = np.empty((B, C, H, W), dtype=np.float32)
    for k in range(N_CORES):
        b, hb = k // 4, k % 4
        # device out [grp, hb, c, r, w] -> [c, hb*16 + grp*4 + r, w]
        o = (
            res.results[k]["o"]
            .astype(np.float32)
            .reshape(NGRP, HB, C, G, W)
            .transpose(2, 1, 0, 3, 4)
            .reshape(C, HL, W)
        )
        out[b, :, hb * HL : (hb + 1) * HL, :] = o
    if _trace:
        return out, res
    return out
